# revision 3
# baseline (speedup 1.0000x reference)
"""Trainium2 Bass kernel for ChunkLayer forward (padded path).

Semantics (per batch row, matching the reference's stable argsort):
  order = [positions where boundary_mask, ascending] ++ [positions where
  ~boundary_mask, ascending]
  next_hidden[j] = hidden[order[j]]           for j < K
  next_mask[j]   = j < num_boundary_tokens

Distribution: data-parallel over batch B=8 -> one row per NeuronCore,
no cross-core communication.

Per-core device algorithm:
  1. Load the row's boundary mask in a "wrapped-by-16" layout [16, L/16]
     (element e at [e%16, e//16]).
  2. Build v[e'] for e' in [0, 2L): first half  v = +(e+1) if mask else -(e+1),
     second half v = -(e+4096+1) if mask else +(e+4096+1)   (DVE).
  3. gpsimd.sparse_gather compacts the negatives out -> the first K slots hold
     (order[j]+1) or (order[j]+4096+1), encoding boundary-ness in the 4096 bit.
  4. PE-transpose the wrapped [16,128] result to natural [128,16] (PSUM),
     then DVE: idx32 = decode position, next_mask = (value <= 4096).
  5. gpsimd.indirect_dma_start gathers the K selected rows (4KB each)
     HBM->SBUF in chunks; HWDGE DMA stores them to the output, double-buffered.
"""

import functools
import numpy as np

from concourse import bass, bacc, mybir, tile
from concourse.bass_utils import run_bass_kernel_spmd

L = 4096  # sequence length per row
D = 1024  # hidden dim
B = 8  # batch == number of cores
FW = L // 16  # wrapped free width for one L-span (256)
F2 = 2 * FW  # both halves (512)
CCH = 4  # columns (=512 tokens) per payload chunk


def _build(k128: int):
    """Build + compile the per-core Bass program for K128 output rows."""
    assert k128 % 128 == 0 and 0 < k128 <= L
    nc = bacc.Bacc(None, target_bir_lowering=False, debug=False)

    hidden = nc.declare_dram_parameter("hidden", [L, D], mybir.dt.float32, isOutput=False)
    maskw = nc.declare_dram_parameter("maskw", [16, FW], mybir.dt.uint8, isOutput=False)
    out_h = nc.declare_dram_parameter("out_h", [k128, D], mybir.dt.float32, isOutput=True)
    out_m = nc.declare_dram_parameter("out_m", [k128], mybir.dt.uint8, isOutput=True)

    # Constants embedded in the NEFF.
    iota_np = (np.arange(2 * L).reshape(F2, 16).T + 1.0).astype(np.float32)
    iota_c = nc.inline_tensor(iota_np, name="iota_ep1")
    ident_c = nc.inline_tensor(np.eye(16, dtype=np.float32), name="ident16")

    Op = mybir.AluOpType

    with tile.TileContext(nc) as tc:
        with (
            tc.tile_pool(name="small", bufs=1) as sp,
            tc.tile_pool(name="psum", bufs=1, space="PSUM") as pp,
            tc.tile_pool(name="pay", bufs=3) as payp,
        ):
            mask_t = sp.tile([16, FW], mybir.dt.uint8)
            iota_t = sp.tile([16, F2], mybir.dt.float32)
            id_t = sp.tile([16, 16], mybir.dt.float32)
            nc.sync.dma_start(out=mask_t[:], in_=maskw[:])
            nc.sync.dma_start(out=iota_t[:], in_=iota_c[:])
            nc.sync.dma_start(out=id_t[:], in_=ident_c[:])

            # Sign tile: +1 where the element survives compaction, -1 where not.
            s_t = sp.tile([16, F2], mybir.dt.float32)
            nc.vector.tensor_scalar(s_t[:, 0:FW], mask_t[:], 2.0, -1.0, Op.mult, Op.add)
            nc.vector.tensor_scalar(s_t[:, FW:F2], mask_t[:], -2.0, 1.0, Op.mult, Op.add)
            v_t = sp.tile([16, F2], mybir.dt.float32)
            nc.vector.tensor_tensor(out=v_t[:], in0=s_t[:], in1=iota_t[:], op=Op.mult)

            # Stable compaction: first L outputs = order[] encoded as e+1.
            cidx_t = sp.tile([16, FW], mybir.dt.float32)
            nf_t = sp.tile([1, 1], mybir.dt.uint32)
            nc.gpsimd.sparse_gather(cidx_t[:], v_t[:], num_found=nf_t[:])

            gcols = k128 // 16  # wrapped columns holding the first k128 slots

            # Decode the wrapped code w = pos+1 (boundary) or pos+4097 (tail)
            # into gather indices, still in wrapped-16 layout.
            tw_t = sp.tile([16, gcols], mybir.dt.float32)
            nc.vector.tensor_scalar(
                tw_t[:], cidx_t[:, 0:gcols], 4097.0, -4096.0, Op.is_ge, Op.mult
            )
            idxw_t = sp.tile([16, gcols], mybir.dt.int16)
            nc.vector.scalar_tensor_tensor(
                idxw_t[:], tw_t[:], -1.0, cidx_t[:, 0:gcols], Op.add, Op.add
            )
            # dma_gather wants the wrapped idxs replicated in all 8 Q7 groups.
            idxr_t = sp.tile([128, gcols], mybir.dt.int16)
            for q in range(8):
                nc.sync.dma_start(out=idxr_t[16 * q : 16 * q + 16, :], in_=idxw_t[:])

            # next_mask: transpose the codes to natural order, compare.
            for g0 in range(0, gcols, 128):
                cols = min(128, gcols - g0)
                ps_t = pp.tile([128, 16], mybir.dt.float32)
                nc.tensor.transpose(ps_t[:cols, :], cidx_t[:, g0 : g0 + cols], id_t[:])
                nm_t = sp.tile([128, 16], mybir.dt.uint8)
                nc.vector.tensor_scalar(nm_t[:cols], ps_t[:cols], 4097.0, None, Op.is_lt)
                nc.sync.dma_start(
                    out=out_m[g0 * 16 : g0 * 16 + cols * 16].rearrange(
                        "(p c) -> p c", c=16
                    ),
                    in_=nm_t[:cols],
                )

            # Payload: gather 512 tokens per dma_gather, store double-buffered.
            # dma_gather writes token j to [j%128, j//128, :].
            CH = 512
            for j0 in range(0, k128, CH):
                n = min(CH, k128 - j0)
                ncols = n // 128
                pay_t = payp.tile([128, CH // 128, D], mybir.dt.float32, tag="pay")
                nc.gpsimd.dma_gather(
                    pay_t[:, 0:ncols, :],
                    hidden[:],
                    idxr_t[:, j0 // 16 : (j0 + n) // 16],
                    n,
                    n,
                    D,
                )
                nc.sync.dma_start(
                    out=out_h[j0 : j0 + n, :].rearrange("(c p) d -> p c d", p=128),
                    in_=pay_t[:, 0:ncols, :],
                )

    nc.compile()
    return nc


@functools.lru_cache(maxsize=4)
def _built(k128: int):
    return _build(k128)


def _marshal_inputs(hidden_states, boundary_mask):
    """Per-core input dicts: row b of hidden + its wrapped mask."""
    in_maps = []
    for b in range(B):
        maskw = (
            np.ascontiguousarray(
                boundary_mask[b].astype(np.uint8).reshape(FW, 16).T
            )
        )
        in_maps.append(
            {
                "hidden": np.ascontiguousarray(hidden_states[b], dtype=np.float32),
                "maskw": maskw,
            }
        )
    return in_maps


def kernel(hidden_states, boundary_mask, mask, next_max_seqlen, _trace=False):
    hidden_states = np.asarray(hidden_states)
    boundary_mask = np.asarray(boundary_mask)
    assert hidden_states.shape == (B, L, D), hidden_states.shape
    assert boundary_mask.shape == (B, L), boundary_mask.shape
    K = int(next_max_seqlen)
    assert 0 < K <= L
    k128 = ((K + 127) // 128) * 128

    nc = _built(k128)
    in_maps = _marshal_inputs(hidden_states, boundary_mask)
    res = run_bass_kernel_spmd(nc, in_maps, list(range(B)), trace=_trace)

    next_hidden = np.stack([res.results[b]["out_h"][:K] for b in range(B)])
    next_mask = np.stack([res.results[b]["out_m"][:K] for b in range(B)]).astype(bool)
    if _trace:
        return (next_hidden, next_mask), res
    return next_hidden, next_mask


# revision 4
# speedup vs baseline: 1.0222x; 1.0222x over previous
"""Trainium2 Bass kernel for ChunkLayer forward (padded path).

Semantics (per batch row, matching the reference's stable argsort):
  order = [positions where boundary_mask, ascending] ++ [positions where
  ~boundary_mask, ascending]
  next_hidden[j] = hidden[order[j]]           for j < K
  next_mask[j]   = j < num_boundary_tokens

Distribution: data-parallel over batch B=8 -> one row per NeuronCore,
no cross-core communication.

Per-core device algorithm:
  1. Load the row's boundary mask in a "wrapped-by-16" layout [16, L/16]
     (element e at [e%16, e//16]).
  2. Build v[e'] for e' in [0, 2L): first half  v = +-(e+1) by mask,
     second half v = -+(e+4096+1) by mask   (DVE, int16).
  3. gpsimd.sparse_gather compacts the negatives out -> the first K slots
     hold (order[j]+1) or (order[j]+4097), encoding boundary-ness in bit 12.
  4. DVE decodes gather indices in wrapped layout; 8 small DMAs replicate
     them to all Q7 groups; meanwhile the gpsimd library switches to `mlp`
     (reload overlaps with the decode/replication).
  5. gpsimd.dma_gather pulls the K selected 4KB rows HBM->SBUF in 512-token
     chunks; HWDGE DMAs store them to the output, quad-buffered.
  6. next_mask: PE-transposes the boundary-flag tile to natural order.
"""

import functools
import numpy as np

from concourse import bacc, library_config, mybir, tile
from concourse.tile_rust import add_dep_helper

L = 4096  # sequence length per row
D = 1024  # hidden dim
B = 8  # batch == number of cores
FW = L // 16  # wrapped free width for one L-span (256)
F2 = 2 * FW  # both halves (512)


def _build(k128: int):
    """Build + compile the per-core Bass program for K128 output rows."""
    assert k128 % 128 == 0 and 0 < k128 <= L
    nc = bacc.Bacc(None, target_bir_lowering=False, debug=False)

    hidden = nc.declare_dram_parameter("hidden", [L, D], mybir.dt.float32, isOutput=False)
    maskw = nc.declare_dram_parameter("maskw", [16, FW], mybir.dt.uint8, isOutput=False)
    out_h = nc.declare_dram_parameter("out_h", [k128, D], mybir.dt.float32, isOutput=True)
    out_m = nc.declare_dram_parameter("out_m", [k128], mybir.dt.uint8, isOutput=True)

    # Constants embedded in the NEFF: wrapped iota (e+1) and a 16x16 identity.
    iota_np = (np.arange(2 * L).reshape(F2, 16).T + 1).astype(np.int16)
    iota_c = nc.inline_tensor(iota_np, name="iota_ep1")
    ident_c = nc.inline_tensor(np.eye(16, dtype=np.float32), name="ident16")

    Op = mybir.AluOpType

    with tile.TileContext(nc) as tc:
        with (
            tc.tile_pool(name="small", bufs=1) as sp,
            tc.tile_pool(name="psum", bufs=1, space="PSUM") as pp,
            tc.tile_pool(name="pay", bufs=4) as payp,
        ):
            mask_t = sp.tile([16, FW], mybir.dt.uint8)
            iota_t = sp.tile([16, F2], mybir.dt.int16)
            id_t = sp.tile([16, 16], mybir.dt.float32)
            nc.sync.dma_start(out=mask_t[:], in_=maskw[:])
            nc.sync.dma_start(out=iota_t[:], in_=iota_c[:])
            nc.sync.dma_start(out=id_t[:], in_=ident_c[:])

            # Sign tile: +1 where the element survives compaction, -1 where not.
            s_t = sp.tile([16, F2], mybir.dt.int16)
            nc.vector.tensor_scalar(s_t[:, 0:FW], mask_t[:], 2, -1, Op.mult, Op.add)
            nc.vector.tensor_scalar(s_t[:, FW:F2], mask_t[:], -2, 1, Op.mult, Op.add)
            v_t = sp.tile([16, F2], mybir.dt.int16)
            nc.vector.tensor_tensor(out=v_t[:], in0=s_t[:], in1=iota_t[:], op=Op.mult)

            # Stable compaction: first L outputs = order[] encoded as code e+1.
            cidx_t = sp.tile([16, FW], mybir.dt.int16)
            nf_t = sp.tile([1, 1], mybir.dt.uint32)
            sg = nc.gpsimd.sparse_gather(cidx_t[:], v_t[:], num_found=nf_t[:])

            # Switch the Q7 library to `mlp` (for dma_gather) right after the
            # compaction so the reload overlaps with decode + replication.
            rl = nc.gpsimd.load_library(library_config.mlp)
            add_dep_helper(rl.ins, sg.ins, sync=False, reason="reload after sparse")

            gcols = k128 // 16  # wrapped columns holding the first k128 slots

            # Decode code w = pos+1 (boundary) or pos+4097 (tail) into gather
            # indices, still in wrapped-16 layout. tw = -4096 where tail.
            tw_t = sp.tile([16, gcols], mybir.dt.float32)
            nc.vector.tensor_scalar(
                tw_t[:], cidx_t[:, 0:gcols], 4097, -4096, Op.is_ge, Op.mult
            )
            idxw_t = sp.tile([16, gcols], mybir.dt.int16)
            nc.vector.scalar_tensor_tensor(
                idxw_t[:], tw_t[:], -1.0, cidx_t[:, 0:gcols], Op.add, Op.add
            )
            # dma_gather wants the wrapped idxs replicated in all 8 Q7 groups;
            # split the copies across both HWDGE engines.
            idxr_t = sp.tile([128, gcols], mybir.dt.int16)
            for q in range(8):
                eng = nc.sync if q % 2 == 0 else nc.scalar
                eng.dma_start(out=idxr_t[16 * q : 16 * q + 16, :], in_=idxw_t[:])

            # next_mask: transpose the boundary-flag tile to natural order.
            for g0 in range(0, gcols, 128):
                cols = min(128, gcols - g0)
                ps_t = pp.tile([128, 16], mybir.dt.float32)
                nc.tensor.transpose(ps_t[:cols, :], tw_t[:, g0 : g0 + cols], id_t[:])
                nm_t = sp.tile([128, 16], mybir.dt.uint8)
                nc.vector.tensor_scalar(nm_t[:cols], ps_t[:cols], 0.0, None, Op.is_equal)
                nc.sync.dma_start(
                    out=out_m[g0 * 16 : g0 * 16 + cols * 16].rearrange(
                        "(p c) -> p c", c=16
                    ),
                    in_=nm_t[:cols],
                )

            # Payload: gather 512 tokens per dma_gather, store quad-buffered.
            # dma_gather writes token j to [j%128, j//128, :].
            CH = 512
            first_gather = None
            for j0 in range(0, k128, CH):
                n = min(CH, k128 - j0)
                ncols = n // 128
                pay_t = payp.tile([128, CH // 128, D], mybir.dt.float32, tag="pay")
                g = nc.gpsimd.dma_gather(
                    pay_t[:, 0:ncols, :],
                    hidden[:],
                    idxr_t[:, j0 // 16 : (j0 + n) // 16],
                    n,
                    n,
                    D,
                )
                if first_gather is None:
                    first_gather = g
                    add_dep_helper(
                        g.ins, rl.ins, sync=False, reason="gather after reload"
                    )
                nc.sync.dma_start(
                    out=out_h[j0 : j0 + n, :].rearrange("(c p) d -> p c d", p=128),
                    in_=pay_t[:, 0:ncols, :],
                )

    nc.compile()
    return nc


@functools.lru_cache(maxsize=4)
def _built(k128: int):
    return _build(k128)


def _marshal_inputs(hidden_states, boundary_mask):
    """Per-core input dicts: row b of hidden + its wrapped mask."""
    in_maps = []
    for b in range(B):
        maskw = np.ascontiguousarray(
            boundary_mask[b].astype(np.uint8).reshape(FW, 16).T
        )
        in_maps.append(
            {
                "hidden": np.ascontiguousarray(hidden_states[b], dtype=np.float32),
                "maskw": maskw,
            }
        )
    return in_maps


def kernel(hidden_states, boundary_mask, mask, next_max_seqlen, _trace=False):
    from concourse.bass_utils import run_bass_kernel_spmd

    hidden_states = np.asarray(hidden_states)
    boundary_mask = np.asarray(boundary_mask)
    assert hidden_states.shape == (B, L, D), hidden_states.shape
    assert boundary_mask.shape == (B, L), boundary_mask.shape
    K = int(next_max_seqlen)
    assert 0 < K <= L
    k128 = ((K + 127) // 128) * 128

    nc = _built(k128)
    in_maps = _marshal_inputs(hidden_states, boundary_mask)
    res = run_bass_kernel_spmd(nc, in_maps, list(range(B)), trace=_trace)

    next_hidden = np.stack([res.results[b]["out_h"][:K] for b in range(B)])
    next_mask = np.stack([res.results[b]["out_m"][:K] for b in range(B)]).astype(bool)
    if _trace:
        return (next_hidden, next_mask), res
    return next_hidden, next_mask


# revision 5
# speedup vs baseline: 1.0593x; 1.0362x over previous
"""Trainium2 Bass kernel for ChunkLayer forward (padded path).

Semantics (per batch row, matching the reference's stable argsort):
  order = [positions where boundary_mask, ascending] ++ [positions where
  ~boundary_mask, ascending]
  next_hidden[j] = hidden[order[j]]           for j < K
  next_mask[j]   = j < num_boundary_tokens

Distribution: data-parallel over batch B=8 -> one row per NeuronCore,
no cross-core communication.

Per-core device algorithm:
  1. Load the row's boundary mask in a "wrapped-by-16" layout [16, L/16]
     (element e at [e%16, e//16]).
  2. Build v[e'] for e' in [0, 2L): first half  v = +-(e+1) by mask,
     second half v = -+(e+4096+1) by mask   (DVE, int16).
  3. gpsimd.sparse_gather compacts the negatives out -> the first K slots
     hold (order[j]+1) or (order[j]+4097), encoding boundary-ness in bit 12.
  4. DVE decodes gather indices in wrapped layout; 8 small DMAs replicate
     them to all Q7 groups; meanwhile the gpsimd library switches to `mlp`
     (reload overlaps with the decode/replication).
  5. gpsimd.dma_gather pulls the K selected 4KB rows HBM->SBUF in 512-token
     chunks; HWDGE DMAs store them to the output, quad-buffered.
  6. next_mask: PE-transposes the boundary-flag tile to natural order.
"""

import functools
import numpy as np

from concourse import bacc, library_config, mybir, tile
from concourse.tile_rust import add_dep_helper

L = 4096  # sequence length per row
D = 1024  # hidden dim
B = 8  # batch == number of cores
FW = L // 16  # wrapped free width for one L-span (256)
F2 = 2 * FW  # both halves (512)


def _build(k128: int):
    """Build + compile the per-core Bass program for K128 output rows."""
    assert k128 % 128 == 0 and 0 < k128 <= L
    nc = bacc.Bacc(
        None, target_bir_lowering=False, debug=False, enable_partition_id=False
    )

    hidden = nc.declare_dram_parameter("hidden", [L, D], mybir.dt.float32, isOutput=False)
    maskw = nc.declare_dram_parameter("maskw", [16, FW], mybir.dt.uint8, isOutput=False)
    out_h = nc.declare_dram_parameter("out_h", [k128, D], mybir.dt.float32, isOutput=True)
    out_m = nc.declare_dram_parameter("out_m", [k128], mybir.dt.uint8, isOutput=True)

    # Constants embedded in the NEFF: wrapped iota (e+1) and a 16x16 identity.
    iota_np = (np.arange(2 * L).reshape(F2, 16).T + 1).astype(np.int16)
    iota_c = nc.inline_tensor(iota_np, name="iota_ep1")
    ident_c = nc.inline_tensor(np.eye(16, dtype=np.float32), name="ident16")

    Op = mybir.AluOpType

    with tile.TileContext(nc) as tc:
        with (
            tc.tile_pool(name="small", bufs=1) as sp,
            tc.tile_pool(name="psum", bufs=1, space="PSUM") as pp,
            tc.tile_pool(name="pay", bufs=4) as payp,
        ):
            mask_t = sp.tile([16, FW], mybir.dt.uint8)
            iota_t = sp.tile([16, F2], mybir.dt.int16)
            id_t = sp.tile([16, 16], mybir.dt.float32)
            nc.sync.dma_start(out=mask_t[:], in_=maskw[:])
            nc.scalar.dma_start(out=iota_t[:], in_=iota_c[:])
            nc.scalar.dma_start(out=id_t[:], in_=ident_c[:])

            # Sign tile: +1 where the element survives compaction, -1 where not.
            s_t = sp.tile([16, F2], mybir.dt.int16)
            nc.vector.tensor_scalar(s_t[:, 0:FW], mask_t[:], 2, -1, Op.mult, Op.add)
            nc.vector.tensor_scalar(s_t[:, FW:F2], mask_t[:], -2, 1, Op.mult, Op.add)
            v_t = sp.tile([16, F2], mybir.dt.int16)
            nc.vector.tensor_tensor(out=v_t[:], in0=s_t[:], in1=iota_t[:], op=Op.mult)

            # Stable compaction: first L outputs = order[] encoded as code e+1.
            cidx_t = sp.tile([16, FW], mybir.dt.int16)
            nf_t = sp.tile([1, 1], mybir.dt.uint32)
            sg = nc.gpsimd.sparse_gather(cidx_t[:], v_t[:], num_found=nf_t[:])

            # Switch the Q7 library to `mlp` (for dma_gather) right after the
            # compaction so the reload overlaps with decode + replication.
            rl = nc.gpsimd.load_library(library_config.mlp)
            add_dep_helper(rl.ins, sg.ins, sync=False, reason="reload after sparse")

            gcols = k128 // 16  # wrapped columns holding the first k128 slots

            # Decode code w = pos+1 (boundary) or pos+4097 (tail) into gather
            # indices, still in wrapped-16 layout. tw = -4096 where tail.
            tw_t = sp.tile([16, gcols], mybir.dt.float32)
            nc.vector.tensor_scalar(
                tw_t[:], cidx_t[:, 0:gcols], 4097, -4096, Op.is_ge, Op.mult
            )
            idxw_t = sp.tile([16, gcols], mybir.dt.int16)
            nc.vector.scalar_tensor_tensor(
                idxw_t[:], tw_t[:], -1.0, cidx_t[:, 0:gcols], Op.add, Op.add
            )
            # dma_gather wants the wrapped idxs replicated in all 8 Q7 groups;
            # split the copies across both HWDGE engines.
            idxr_t = sp.tile([128, gcols], mybir.dt.int16)
            for q in range(8):
                eng = nc.sync if q % 2 == 0 else nc.scalar
                eng.dma_start(out=idxr_t[16 * q : 16 * q + 16, :], in_=idxw_t[:])

            # next_mask: transpose the boundary-flag tile to natural order.
            for g0 in range(0, gcols, 128):
                cols = min(128, gcols - g0)
                ps_t = pp.tile([128, 16], mybir.dt.float32)
                nc.tensor.transpose(ps_t[:cols, :], tw_t[:, g0 : g0 + cols], id_t[:])
                nm_t = sp.tile([128, 16], mybir.dt.uint8)
                nc.vector.tensor_scalar(nm_t[:cols], ps_t[:cols], 0.0, None, Op.is_equal)
                nc.sync.dma_start(
                    out=out_m[g0 * 16 : g0 * 16 + cols * 16].rearrange(
                        "(p c) -> p c", c=16
                    ),
                    in_=nm_t[:cols],
                )

            # Payload: ramped chunk sizes so the first gather's descriptor
            # generation doesn't delay the DMA stream; alternate the store
            # engine so both HWDGE rings drain in parallel with the gathers.
            chunks = []
            left, j0 = k128, 0
            for n in [128, 384]:
                if left >= n + 512 or left == n:
                    chunks.append(n)
                    left -= n
            while left:
                n = min(512, left)
                chunks.append(n)
                left -= n
            first_gather = None
            for ci, n in enumerate(chunks):
                ncols = n // 128
                pay_t = payp.tile([128, 4, D], mybir.dt.float32, tag="pay")
                g = nc.gpsimd.dma_gather(
                    pay_t[:, 0:ncols, :],
                    hidden[:],
                    idxr_t[:, j0 // 16 : (j0 + n) // 16],
                    n,
                    n,
                    D,
                    single_packet=False,
                )
                if first_gather is None:
                    first_gather = g
                    add_dep_helper(
                        g.ins, rl.ins, sync=False, reason="gather after reload"
                    )
                seng = nc.sync if ci % 2 == 0 else nc.scalar
                seng.dma_start(
                    out=out_h[j0 : j0 + n, :].rearrange("(c p) d -> p c d", p=128),
                    in_=pay_t[:, 0:ncols, :],
                )
                j0 += n

    nc.compile()
    return nc


@functools.lru_cache(maxsize=4)
def _built(k128: int):
    return _build(k128)


def _marshal_inputs(hidden_states, boundary_mask):
    """Per-core input dicts: row b of hidden + its wrapped mask."""
    in_maps = []
    for b in range(B):
        maskw = np.ascontiguousarray(
            boundary_mask[b].astype(np.uint8).reshape(FW, 16).T
        )
        in_maps.append(
            {
                "hidden": np.ascontiguousarray(hidden_states[b], dtype=np.float32),
                "maskw": maskw,
            }
        )
    return in_maps


def kernel(hidden_states, boundary_mask, mask, next_max_seqlen, _trace=False):
    from concourse.bass_utils import run_bass_kernel_spmd

    hidden_states = np.asarray(hidden_states)
    boundary_mask = np.asarray(boundary_mask)
    assert hidden_states.shape == (B, L, D), hidden_states.shape
    assert boundary_mask.shape == (B, L), boundary_mask.shape
    K = int(next_max_seqlen)
    assert 0 < K <= L
    k128 = ((K + 127) // 128) * 128

    nc = _built(k128)
    in_maps = _marshal_inputs(hidden_states, boundary_mask)
    res = run_bass_kernel_spmd(nc, in_maps, list(range(B)), trace=_trace)

    next_hidden = np.stack([res.results[b]["out_h"][:K] for b in range(B)])
    next_mask = np.stack([res.results[b]["out_m"][:K] for b in range(B)]).astype(bool)
    if _trace:
        return (next_hidden, next_mask), res
    return next_hidden, next_mask


# revision 6
# speedup vs baseline: 1.1408x; 1.0769x over previous
"""Trainium2 Bass kernel for ChunkLayer forward (padded path).

Semantics (per batch row, matching the reference's stable argsort):
  order = [positions where boundary_mask, ascending] ++ [positions where
  ~boundary_mask, ascending]
  next_hidden[j] = hidden[order[j]]           for j < K
  next_mask[j]   = j < num_boundary_tokens

Distribution: data-parallel over batch B=8 -> one row per NeuronCore,
no cross-core communication.

Per-core device algorithm:
  1. Load the row's boundary mask in a "wrapped-by-16" layout [16, L/16]
     (element e at [e%16, e//16]).
  2. Build v[e'] for e' in [0, 2L): first half  v = +-(e+1) by mask,
     second half v = -+(e+4096+1) by mask   (DVE, int16).
  3. gpsimd.sparse_gather compacts the negatives out -> the first K slots
     hold (order[j]+1) or (order[j]+4097), encoding boundary-ness in bit 12.
  4. DVE decodes gather indices in wrapped layout; 8 small DMAs replicate
     them to all Q7 groups; meanwhile the gpsimd library switches to `mlp`
     (reload overlaps with the decode/replication).
  5. gpsimd.dma_gather pulls the K selected 4KB rows HBM->SBUF in 512-token
     chunks; HWDGE DMAs store them to the output, quad-buffered.
  6. next_mask: PE-transposes the boundary-flag tile to natural order.
"""

import functools
import numpy as np

from concourse import bacc, library_config, mybir, tile
from concourse.tile_rust import add_dep_helper

L = 4096  # sequence length per row
D = 1024  # hidden dim
B = 8  # batch == number of cores
FW = L // 16  # wrapped free width for one L-span (256)
F2 = 2 * FW  # both halves (512)


def _build(k128: int):
    """Build + compile the per-core Bass program for K128 output rows."""
    assert k128 % 128 == 0 and 0 < k128 <= L
    nc = bacc.Bacc(
        None, target_bir_lowering=False, debug=False, enable_partition_id=False
    )

    hidden = nc.declare_dram_parameter("hidden", [L, D], mybir.dt.float32, isOutput=False)
    maskw = nc.declare_dram_parameter("maskw", [16, FW], mybir.dt.uint8, isOutput=False)
    out_h = nc.declare_dram_parameter("out_h", [k128, D], mybir.dt.float32, isOutput=True)
    out_m = nc.declare_dram_parameter("out_m", [k128], mybir.dt.uint8, isOutput=True)

    # Constants embedded in the NEFF: wrapped iota (e+1) and a 16x16 identity.
    iota_np = (np.arange(2 * L).reshape(F2, 16).T + 1).astype(np.int16)
    iota_c = nc.inline_tensor(iota_np, name="iota_ep1")
    ident_c = nc.inline_tensor(np.eye(16, dtype=np.float32), name="ident16")

    Op = mybir.AluOpType

    with tile.TileContext(nc) as tc:
        with (
            tc.tile_pool(name="small", bufs=1) as sp,
            tc.tile_pool(name="psum", bufs=1, space="PSUM") as pp,
            tc.tile_pool(name="pay", bufs=6) as payp,
        ):
            mask_t = sp.tile([16, FW], mybir.dt.uint8)
            iota_t = sp.tile([16, F2], mybir.dt.int16)
            id_t = sp.tile([16, 16], mybir.dt.float32)
            nc.sync.dma_start(out=mask_t[:], in_=maskw[:])
            nc.scalar.dma_start(out=iota_t[:], in_=iota_c[:])
            nc.scalar.dma_start(out=id_t[:], in_=ident_c[:])

            # Sign tile: +1 where the element survives compaction, -1 where not.
            s_t = sp.tile([16, F2], mybir.dt.int16)
            nc.vector.tensor_scalar(s_t[:, 0:FW], mask_t[:], 2, -1, Op.mult, Op.add)
            nc.vector.tensor_scalar(s_t[:, FW:F2], mask_t[:], -2, 1, Op.mult, Op.add)
            v_t = sp.tile([16, F2], mybir.dt.int16)
            nc.vector.tensor_tensor(out=v_t[:], in0=s_t[:], in1=iota_t[:], op=Op.mult)

            # Stable compaction: first L outputs = order[] encoded as code e+1.
            cidx_t = sp.tile([16, FW], mybir.dt.int16)
            nf_t = sp.tile([1, 1], mybir.dt.uint32)
            sg = nc.gpsimd.sparse_gather(cidx_t[:], v_t[:], num_found=nf_t[:])

            # Switch the Q7 library to `mlp` (for dma_gather) right after the
            # compaction so the reload overlaps with decode + replication.
            rl = nc.gpsimd.load_library(library_config.mlp)
            add_dep_helper(rl.ins, sg.ins, sync=False, reason="reload after sparse")

            gcols = k128 // 16  # wrapped columns holding the first k128 slots

            # Decode code w = pos+1 (boundary) or pos+4097 (tail) into gather
            # indices, still in wrapped-16 layout. tw = -4096 where tail.
            tw_t = sp.tile([16, gcols], mybir.dt.float32)
            nc.vector.tensor_scalar(
                tw_t[:], cidx_t[:, 0:gcols], 4097, -4096, Op.is_ge, Op.mult
            )
            idxw_t = sp.tile([16, gcols], mybir.dt.int16)
            nc.vector.scalar_tensor_tensor(
                idxw_t[:], tw_t[:], -1.0, cidx_t[:, 0:gcols], Op.add, Op.add
            )
            # dma_gather wants the wrapped idxs replicated in all 8 Q7 groups;
            # split the copies across both HWDGE engines.
            idxr_t = sp.tile([128, gcols], mybir.dt.int16)
            for q in range(8):
                eng = nc.sync if q % 2 == 0 else nc.scalar
                eng.dma_start(out=idxr_t[16 * q : 16 * q + 16, :], in_=idxw_t[:])

            # next_mask: transpose the boundary-flag tile to natural order.
            for g0 in range(0, gcols, 128):
                cols = min(128, gcols - g0)
                ps_t = pp.tile([128, 16], mybir.dt.float32)
                nc.tensor.transpose(ps_t[:cols, :], tw_t[:, g0 : g0 + cols], id_t[:])
                nm_t = sp.tile([128, 16], mybir.dt.uint8)
                nc.vector.tensor_scalar(nm_t[:cols], ps_t[:cols], 0.0, None, Op.is_equal)
                nc.sync.dma_start(
                    out=out_m[g0 * 16 : g0 * 16 + cols * 16].rearrange(
                        "(p c) -> p c", c=16
                    ),
                    in_=nm_t[:cols],
                )

            # Payload: chunk sizes ramp up at the start (so the first
            # gather's descriptor generation doesn't delay the DMA stream)
            # and down at the end (so the last store's latency is short).
            # Each chunk's store is split across both HWDGE rings.
            if k128 >= 2048:
                chunks = [128, 384] + [512] * ((k128 - 1024) // 512) + [384, 128]
                rem = k128 - sum(chunks)
                if rem:
                    chunks.insert(2, rem)
            else:
                chunks = []
                left = k128
                while left:
                    n = min(512, left)
                    chunks.append(n)
                    left -= n
            assert sum(chunks) == k128 and all(c % 128 == 0 for c in chunks)
            first_gather = None
            j0 = 0
            for ci, n in enumerate(chunks):
                ncols = n // 128
                pay_t = payp.tile([128, 4, D], mybir.dt.float32, tag="pay")
                g = nc.gpsimd.dma_gather(
                    pay_t[:, 0:ncols, :],
                    hidden[:],
                    idxr_t[:, j0 // 16 : (j0 + n) // 16],
                    n,
                    n,
                    D,
                )
                if first_gather is None:
                    first_gather = g
                    add_dep_helper(
                        g.ins, rl.ins, sync=False, reason="gather after reload"
                    )
                outv = out_h[j0 : j0 + n, :].rearrange("(c p) d -> p c d", p=128)
                if ncols > 1:
                    h = ncols // 2
                    nc.sync.dma_start(out=outv[:, 0:h, :], in_=pay_t[:, 0:h, :])
                    nc.scalar.dma_start(
                        out=outv[:, h:ncols, :], in_=pay_t[:, h:ncols, :]
                    )
                else:
                    eng = nc.sync if ci % 2 == 0 else nc.scalar
                    eng.dma_start(out=outv[:], in_=pay_t[:, 0:ncols, :])
                j0 += n

    nc.compile()
    return nc


@functools.lru_cache(maxsize=4)
def _built(k128: int):
    return _build(k128)


def _marshal_inputs(hidden_states, boundary_mask):
    """Per-core input dicts: row b of hidden + its wrapped mask."""
    in_maps = []
    for b in range(B):
        maskw = np.ascontiguousarray(
            boundary_mask[b].astype(np.uint8).reshape(FW, 16).T
        )
        in_maps.append(
            {
                "hidden": np.ascontiguousarray(hidden_states[b], dtype=np.float32),
                "maskw": maskw,
            }
        )
    return in_maps


def kernel(hidden_states, boundary_mask, mask, next_max_seqlen, _trace=False):
    from concourse.bass_utils import run_bass_kernel_spmd

    hidden_states = np.asarray(hidden_states)
    boundary_mask = np.asarray(boundary_mask)
    assert hidden_states.shape == (B, L, D), hidden_states.shape
    assert boundary_mask.shape == (B, L), boundary_mask.shape
    K = int(next_max_seqlen)
    assert 0 < K <= L
    k128 = ((K + 127) // 128) * 128

    nc = _built(k128)
    in_maps = _marshal_inputs(hidden_states, boundary_mask)
    res = run_bass_kernel_spmd(nc, in_maps, list(range(B)), trace=_trace)

    next_hidden = np.stack([res.results[b]["out_h"][:K] for b in range(B)])
    next_mask = np.stack([res.results[b]["out_m"][:K] for b in range(B)]).astype(bool)
    if _trace:
        return (next_hidden, next_mask), res
    return next_hidden, next_mask


# revision 7
# speedup vs baseline: 1.1775x; 1.0322x over previous
"""Trainium2 Bass kernel for ChunkLayer forward (padded path).

Semantics (per batch row, matching the reference's stable argsort):
  order = [positions where boundary_mask, ascending] ++ [positions where
  ~boundary_mask, ascending]
  next_hidden[j] = hidden[order[j]]           for j < K
  next_mask[j]   = j < num_boundary_tokens

Distribution: data-parallel over batch B=8 -> one row per NeuronCore,
no cross-core communication.

Per-core device algorithm:
  1. Load the row's boundary mask in a "wrapped-by-16" layout [16, L/16]
     (element e at [e%16, e//16]).
  2. Build v[e'] for e' in [0, 2L): first half  v = +-(e+1) by mask,
     second half v = -+(e+4096+1) by mask   (DVE, int16).
  3. gpsimd.sparse_gather compacts the negatives out -> the first K slots
     hold (order[j]+1) or (order[j]+4097), encoding boundary-ness in bit 12.
  4. PE transposes the codes to natural [128, 16] order; DVE decodes
     gather indices + next_mask from them.
  5. gpsimd.indirect_dma_start pulls the selected 4KB rows HBM->SBUF
     (one dynamic offset per partition per call); both HWDGE rings store
     them to the output, deep-buffered.
"""

import functools
import numpy as np

from concourse import bass, bacc, mybir, tile

L = 4096  # sequence length per row
D = 1024  # hidden dim
B = 8  # batch == number of cores
FW = L // 16  # wrapped free width for one L-span (256)
F2 = 2 * FW  # both halves (512)


def _build(k128: int):
    """Build + compile the per-core Bass program for K128 output rows."""
    assert k128 % 128 == 0 and 0 < k128 <= L
    nc = bacc.Bacc(
        None, target_bir_lowering=False, debug=False, enable_partition_id=False
    )

    hidden = nc.declare_dram_parameter("hidden", [L, D], mybir.dt.float32, isOutput=False)
    maskw = nc.declare_dram_parameter("maskw", [16, FW], mybir.dt.uint8, isOutput=False)
    out_h = nc.declare_dram_parameter("out_h", [k128, D], mybir.dt.float32, isOutput=True)
    out_m = nc.declare_dram_parameter("out_m", [k128], mybir.dt.uint8, isOutput=True)

    # Constants embedded in the NEFF: wrapped iota (e+1) and a 16x16 identity.
    iota_np = (np.arange(2 * L).reshape(F2, 16).T + 1).astype(np.int16)
    iota_c = nc.inline_tensor(iota_np, name="iota_ep1")
    ident_c = nc.inline_tensor(np.eye(16, dtype=np.float32), name="ident16")

    Op = mybir.AluOpType

    with tile.TileContext(nc) as tc:
        with (
            tc.tile_pool(name="small", bufs=1) as sp,
            tc.tile_pool(name="psum", bufs=1, space="PSUM") as pp,
            tc.tile_pool(name="pay", bufs=6) as payp,
        ):
            mask_t = sp.tile([16, FW], mybir.dt.uint8)
            iota_t = sp.tile([16, F2], mybir.dt.int16)
            id_t = sp.tile([16, 16], mybir.dt.float32)
            nc.sync.dma_start(out=mask_t[:], in_=maskw[:])
            nc.scalar.dma_start(out=iota_t[:], in_=iota_c[:])
            nc.scalar.dma_start(out=id_t[:], in_=ident_c[:])

            # Sign tile: +1 where the element survives compaction, -1 where not.
            s_t = sp.tile([16, F2], mybir.dt.int16)
            nc.vector.tensor_scalar(s_t[:, 0:FW], mask_t[:], 2, -1, Op.mult, Op.add)
            nc.vector.tensor_scalar(s_t[:, FW:F2], mask_t[:], -2, 1, Op.mult, Op.add)
            v_t = sp.tile([16, F2], mybir.dt.int16)
            nc.vector.tensor_tensor(out=v_t[:], in0=s_t[:], in1=iota_t[:], op=Op.mult)

            # Stable compaction: first L outputs = order[] encoded as code e+1.
            cidx_t = sp.tile([16, FW], mybir.dt.int16)
            nf_t = sp.tile([1, 1], mybir.dt.uint32)
            sg = nc.gpsimd.sparse_gather(cidx_t[:], v_t[:], num_found=nf_t[:])

            gcols = k128 // 16  # wrapped columns holding the first k128 slots

            # Codes to f32 for the PE transpose.
            cidxf_t = sp.tile([16, gcols], mybir.dt.float32)
            nc.vector.tensor_copy(out=cidxf_t[:], in_=cidx_t[:, 0:gcols])

            for g0 in range(0, gcols, 128):
                cols = min(128, gcols - g0)
                tok0 = g0 * 16
                # Natural order: psum[p, c] = code of output row tok0 + 16p + c.
                ps_t = pp.tile([128, 16], mybir.dt.float32)
                nc.tensor.transpose(
                    ps_t[:cols, :], cidxf_t[:, g0 : g0 + cols], id_t[:]
                )
                # Decode code w = pos+1 (boundary) or pos+4097 (tail).
                t1_t = sp.tile([128, 16], mybir.dt.float32)
                nc.vector.tensor_scalar(
                    t1_t[:cols], ps_t[:cols], 4097.0, -4096.0, Op.is_ge, Op.mult
                )
                idx_t = sp.tile([128, 16], mybir.dt.int32, tag=f"idx{g0}")
                nc.vector.scalar_tensor_tensor(
                    idx_t[:cols], t1_t[:cols], -1.0, ps_t[:cols], Op.add, Op.add
                )
                nm_t = sp.tile([128, 16], mybir.dt.uint8)
                nc.vector.tensor_scalar(
                    nm_t[:cols], ps_t[:cols], 4097.0, None, Op.is_lt
                )
                nc.sync.dma_start(
                    out=out_m[tok0 : tok0 + cols * 16].rearrange("(p c) -> p c", c=16),
                    in_=nm_t[:cols],
                )

                # Payload: one indirect DMA per token column (the DGE path
                # supports one dynamic offset per partition); store each
                # 4-column tile split across both HWDGE rings. Row 16p+c of
                # the output gets hidden[idx[p, c]], so each partition's two
                # store columns are 8KB contiguous in HBM.
                outg = out_h[tok0 : tok0 + cols * 16, :].rearrange(
                    "(p c) d -> p c d", c=16
                )
                for t in range(0, 16, 4):
                    pay_t = payp.tile([128, 4, D], mybir.dt.float32, tag="pay")
                    for cc in range(4):
                        nc.gpsimd.indirect_dma_start(
                            out=pay_t[:cols, cc, :],
                            out_offset=None,
                            in_=hidden[:],
                            in_offset=bass.IndirectOffsetOnAxis(
                                ap=idx_t[:cols, t + cc : t + cc + 1], axis=0
                            ),
                        )
                    nc.sync.dma_start(
                        out=outg[:, t : t + 2, :], in_=pay_t[:cols, 0:2, :]
                    )
                    nc.scalar.dma_start(
                        out=outg[:, t + 2 : t + 4, :], in_=pay_t[:cols, 2:4, :]
                    )

    nc.compile()
    return nc


@functools.lru_cache(maxsize=4)
def _built(k128: int):
    return _build(k128)


def _marshal_inputs(hidden_states, boundary_mask):
    """Per-core input dicts: row b of hidden + its wrapped mask."""
    in_maps = []
    for b in range(B):
        maskw = np.ascontiguousarray(
            boundary_mask[b].astype(np.uint8).reshape(FW, 16).T
        )
        in_maps.append(
            {
                "hidden": np.ascontiguousarray(hidden_states[b], dtype=np.float32),
                "maskw": maskw,
            }
        )
    return in_maps


def kernel(hidden_states, boundary_mask, mask, next_max_seqlen, _trace=False):
    from concourse.bass_utils import run_bass_kernel_spmd

    hidden_states = np.asarray(hidden_states)
    boundary_mask = np.asarray(boundary_mask)
    assert hidden_states.shape == (B, L, D), hidden_states.shape
    assert boundary_mask.shape == (B, L), boundary_mask.shape
    K = int(next_max_seqlen)
    assert 0 < K <= L
    k128 = ((K + 127) // 128) * 128

    nc = _built(k128)
    in_maps = _marshal_inputs(hidden_states, boundary_mask)
    res = run_bass_kernel_spmd(nc, in_maps, list(range(B)), trace=_trace)

    next_hidden = np.stack([res.results[b]["out_h"][:K] for b in range(B)])
    next_mask = np.stack([res.results[b]["out_m"][:K] for b in range(B)]).astype(bool)
    if _trace:
        return (next_hidden, next_mask), res
    return next_hidden, next_mask


# revision 8
# speedup vs baseline: 1.1869x; 1.0080x over previous
"""Trainium2 Bass kernel for ChunkLayer forward (padded path).

Semantics (per batch row, matching the reference's stable argsort):
  order = [positions where boundary_mask, ascending] ++ [positions where
  ~boundary_mask, ascending]
  next_hidden[j] = hidden[order[j]]           for j < K
  next_mask[j]   = j < num_boundary_tokens

Distribution: data-parallel over batch B=8 -> one row per NeuronCore,
no cross-core communication.

Per-core device algorithm:
  1. Load the row's boundary mask in a "wrapped-by-16" layout [16, L/16]
     (element e at [e%16, e//16]).
  2. Build v[e'] for e' in [0, 2L): first half  v = +-(e+1) by mask,
     second half v = -+(e+4096+1) by mask   (DVE, int16).
  3. gpsimd.sparse_gather compacts the negatives out -> the first K slots
     hold (order[j]+1) or (order[j]+4097), encoding boundary-ness in bit 12.
  4. PE transposes the codes to natural [128, 16] order; DVE decodes
     gather indices + next_mask from them.
  5. gpsimd.indirect_dma_start pulls the selected 4KB rows HBM->SBUF
     (one dynamic offset per partition per call); both HWDGE rings store
     them to the output, deep-buffered.
"""

import functools
import numpy as np

from concourse import bass, bacc, mybir, tile

L = 4096  # sequence length per row
D = 1024  # hidden dim
B = 8  # batch == number of cores
FW = L // 16  # wrapped free width for one L-span (256)
F2 = 2 * FW  # both halves (512)


def _build(k128: int):
    """Build + compile the per-core Bass program for K128 output rows."""
    assert k128 % 128 == 0 and 0 < k128 <= L
    nc = bacc.Bacc(
        None,
        target_bir_lowering=False,
        debug=False,
        enable_partition_id=False,
        monotonic_sem_count=0,
    )

    hidden = nc.declare_dram_parameter("hidden", [L, D], mybir.dt.float32, isOutput=False)
    maskw = nc.declare_dram_parameter("maskw", [16, FW], mybir.dt.uint8, isOutput=False)
    out_h = nc.declare_dram_parameter("out_h", [k128, D], mybir.dt.float32, isOutput=True)
    out_m = nc.declare_dram_parameter("out_m", [k128], mybir.dt.uint8, isOutput=True)

    # Constants embedded in the NEFF: signed wrapped iotas and an identity.
    # Codes: e+1 for kept elements, -(e+1) for dropped ones; the second half
    # of the concat keeps where the mask is 0.
    ep1 = (np.arange(2 * L).reshape(F2, 16).T + 1).astype(np.int16)
    a_np = np.concatenate([ep1[:, :FW], -ep1[:, FW:]], axis=1)
    b_np = -a_np
    a_c = nc.inline_tensor(a_np, name="keep_codes")
    b_c = nc.inline_tensor(b_np, name="drop_codes")
    ident_c = nc.inline_tensor(np.eye(16, dtype=np.float32), name="ident16")

    Op = mybir.AluOpType

    with tile.TileContext(nc) as tc:
        with (
            tc.tile_pool(name="small", bufs=1) as sp,
            tc.tile_pool(name="psum", bufs=1, space="PSUM") as pp,
            tc.tile_pool(name="pay", bufs=6) as payp,
        ):
            m2_t = sp.tile([16, F2], mybir.dt.uint8)
            a_t = sp.tile([16, F2], mybir.dt.int16)
            b_t = sp.tile([16, F2], mybir.dt.int16)
            id_t = sp.tile([16, 16], mybir.dt.float32)
            nc.sync.dma_start(out=m2_t[:, 0:FW], in_=maskw[:])
            nc.sync.dma_start(out=m2_t[:, FW:F2], in_=maskw[:])
            nc.scalar.dma_start(out=a_t[:], in_=a_c[:])
            nc.scalar.dma_start(out=b_t[:], in_=b_c[:])
            nc.scalar.dma_start(out=id_t[:], in_=ident_c[:])

            # v = keep/drop code by mask: +code survives compaction.
            v_t = sp.tile([16, F2], mybir.dt.int16)
            nc.vector.tensor_copy(out=v_t[:], in_=b_t[:])
            nc.vector.copy_predicated(out=v_t[:], mask=m2_t[:], data=a_t[:])

            # Stable compaction: first L outputs = order[] encoded as code e+1.
            cidx_t = sp.tile([16, FW], mybir.dt.int16)
            nf_t = sp.tile([1, 1], mybir.dt.uint32)
            sg = nc.gpsimd.sparse_gather(cidx_t[:], v_t[:], num_found=nf_t[:])

            gcols = k128 // 16  # wrapped columns holding the first k128 slots

            # Codes to f32 for the PE transpose.
            cidxf_t = sp.tile([16, gcols], mybir.dt.float32)
            nc.vector.tensor_copy(out=cidxf_t[:], in_=cidx_t[:, 0:gcols])

            for g0 in range(0, gcols, 128):
                cols = min(128, gcols - g0)
                tok0 = g0 * 16
                # Natural order: psum[p, c] = code of output row tok0 + 16p + c.
                ps_t = pp.tile([128, 16], mybir.dt.float32)
                nc.tensor.transpose(
                    ps_t[:cols, :], cidxf_t[:, g0 : g0 + cols], id_t[:]
                )
                # Decode code w = pos+1 (boundary) or pos+4097 (tail).
                t1_t = sp.tile([128, 16], mybir.dt.float32)
                nc.vector.tensor_scalar(
                    t1_t[:cols], ps_t[:cols], 4097.0, -4096.0, Op.is_ge, Op.mult
                )
                idx_t = sp.tile([128, 16], mybir.dt.int32, tag=f"idx{g0}")
                nc.vector.scalar_tensor_tensor(
                    idx_t[:cols], t1_t[:cols], -1.0, ps_t[:cols], Op.add, Op.add
                )
                nm_t = sp.tile([128, 16], mybir.dt.uint8)
                nc.vector.tensor_scalar(
                    nm_t[:cols], ps_t[:cols], 4097.0, None, Op.is_lt
                )
                nc.sync.dma_start(
                    out=out_m[tok0 : tok0 + cols * 16].rearrange("(p c) -> p c", c=16),
                    in_=nm_t[:cols],
                )

                # Payload: one indirect DMA per token column (the DGE path
                # supports one dynamic offset per partition); store each
                # 4-column tile split across both HWDGE rings. Row 16p+c of
                # the output gets hidden[idx[p, c]], so each partition's two
                # store columns are 8KB contiguous in HBM.
                outg = out_h[tok0 : tok0 + cols * 16, :].rearrange(
                    "(p c) d -> p c d", c=16
                )
                for t in range(0, 16, 4):
                    pay_t = payp.tile([128, 4, D], mybir.dt.float32, tag="pay")
                    for cc in range(4):
                        nc.gpsimd.indirect_dma_start(
                            out=pay_t[:cols, cc, :],
                            out_offset=None,
                            in_=hidden[:],
                            in_offset=bass.IndirectOffsetOnAxis(
                                ap=idx_t[:cols, t + cc : t + cc + 1], axis=0
                            ),
                        )
                        seng = nc.sync if cc % 2 == 0 else nc.scalar
                        seng.dma_start(
                            out=outg[:, t + cc : t + cc + 1, :],
                            in_=pay_t[:cols, cc : cc + 1, :],
                        )

    nc.compile()
    return nc


@functools.lru_cache(maxsize=4)
def _built(k128: int):
    return _build(k128)


def _marshal_inputs(hidden_states, boundary_mask):
    """Per-core input dicts: row b of hidden + its wrapped mask."""
    in_maps = []
    for b in range(B):
        maskw = np.ascontiguousarray(
            boundary_mask[b].astype(np.uint8).reshape(FW, 16).T
        )
        in_maps.append(
            {
                "hidden": np.ascontiguousarray(hidden_states[b], dtype=np.float32),
                "maskw": maskw,
            }
        )
    return in_maps


def kernel(hidden_states, boundary_mask, mask, next_max_seqlen, _trace=False):
    from concourse.bass_utils import run_bass_kernel_spmd

    hidden_states = np.asarray(hidden_states)
    boundary_mask = np.asarray(boundary_mask)
    assert hidden_states.shape == (B, L, D), hidden_states.shape
    assert boundary_mask.shape == (B, L), boundary_mask.shape
    K = int(next_max_seqlen)
    assert 0 < K <= L
    k128 = ((K + 127) // 128) * 128

    nc = _built(k128)
    in_maps = _marshal_inputs(hidden_states, boundary_mask)
    res = run_bass_kernel_spmd(nc, in_maps, list(range(B)), trace=_trace)

    next_hidden = np.stack([res.results[b]["out_h"][:K] for b in range(B)])
    next_mask = np.stack([res.results[b]["out_m"][:K] for b in range(B)]).astype(bool)
    if _trace:
        return (next_hidden, next_mask), res
    return next_hidden, next_mask


# revision 10
# speedup vs baseline: 1.2850x; 1.0826x over previous
"""Trainium2 Bass kernel for ChunkLayer forward (padded path).

Semantics (per batch row, matching the reference's stable argsort):
  order = [positions where boundary_mask, ascending] ++ [positions where
  ~boundary_mask, ascending]
  next_hidden[j] = hidden[order[j]]           for j < K
  next_mask[j]   = j < num_boundary_tokens

Distribution: data-parallel over batch B=8 -> one row per NeuronCore,
no cross-core communication.

Per-core device algorithm:
  1. Load the row's boundary mask in a "wrapped-by-16" layout [16, L/16]
     (element e at [e%16, e//16]).
  2. Build v[e'] for e' in [0, 2L): first half  v = +-(e+1) by mask,
     second half v = -+(e+4096+1) by mask   (DVE, int16).
  3. gpsimd.sparse_gather compacts the negatives out -> the first K slots
     hold (order[j]+1) or (order[j]+4097), encoding boundary-ness in bit 12.
  4. PE transposes the codes to natural [128, 16] order; DVE decodes
     gather indices + next_mask from them.
  5. gpsimd.indirect_dma_start pulls the selected 4KB rows HBM->SBUF
     (one dynamic offset per partition per call); both HWDGE rings store
     them to the output, deep-buffered.
"""

import functools
import sys
import types

import numpy as np

from concourse import bass, bacc, mybir, tile


def _ensure_axon_ntff_hook():
    """This container's antenv package lacks axon_hooks; bass_utils imports it
    unconditionally when tracing is requested (including via BASS_TRACE=1).
    Install a small shim and register the real NTFF hook when available."""
    try:
        import antenv.axon_hooks  # noqa: F401
        return
    except ImportError:
        pass
    try:
        import antenv
    except ImportError:
        return
    mod = types.ModuleType("antenv.axon_hooks")
    mod._hook = None
    mod.set_axon_ntff_profile_hook = lambda h: setattr(mod, "_hook", h)
    mod.get_axon_ntff_profile_hook = lambda: mod._hook
    sys.modules["antenv.axon_hooks"] = mod
    antenv.axon_hooks = mod
    try:
        from trn_agent_boot.trn_boot import _ntff_profile_via_ctypes

        mod._hook = _ntff_profile_via_ctypes("/opt/axon/libaxon_pjrt.so")
    except Exception:
        pass


_ensure_axon_ntff_hook()

L = 4096  # sequence length per row
D = 1024  # hidden dim
B = 8  # batch == number of cores
FW = L // 16  # wrapped free width for one L-span (256)
F2 = 2 * FW  # both halves (512)


def _build(k128: int):
    """Build + compile the per-core Bass program for K128 output rows."""
    assert k128 % 128 == 0 and 0 < k128 <= L
    nc = bacc.Bacc(
        None,
        target_bir_lowering=False,
        debug=False,
        enable_partition_id=False,
        monotonic_sem_count=0,
        dynamic_dma_scratch_size=32768,
    )

    hidden = nc.declare_dram_parameter("hidden", [L, D], mybir.dt.float32, isOutput=False)
    maskw = nc.declare_dram_parameter("maskw", [16, FW], mybir.dt.uint8, isOutput=False)
    out_h = nc.declare_dram_parameter("out_h", [k128, D], mybir.dt.float32, isOutput=True)
    out_m = nc.declare_dram_parameter("out_m", [k128], mybir.dt.uint8, isOutput=True)

    # Constants embedded in the NEFF: signed wrapped iotas and an identity.
    # Codes: e+1 for kept elements, -(e+1) for dropped ones; the second half
    # of the concat keeps where the mask is 0.
    ep1 = (np.arange(2 * L).reshape(F2, 16).T + 1).astype(np.int16)
    a_np = np.concatenate([ep1[:, :FW], -ep1[:, FW:]], axis=1)
    b_np = -a_np
    a_c = nc.inline_tensor(a_np, name="keep_codes")
    b_c = nc.inline_tensor(b_np, name="drop_codes")
    ident_c = nc.inline_tensor(np.eye(16, dtype=np.float32), name="ident16")

    Op = mybir.AluOpType

    with tile.TileContext(nc) as tc:
        with (
            tc.tile_pool(name="small", bufs=1) as sp,
            tc.tile_pool(name="psum", bufs=1, space="PSUM") as pp,
            tc.tile_pool(name="pay", bufs=6) as payp,
        ):
            m2_t = sp.tile([16, F2], mybir.dt.uint8)
            a_t = sp.tile([16, F2], mybir.dt.int16)
            b_t = sp.tile([16, F2], mybir.dt.int16)
            id_t = sp.tile([16, 16], mybir.dt.float32)
            nc.scalar.dma_start(out=b_t[:], in_=b_c[:])
            nc.sync.dma_start(out=m2_t[:, 0:FW], in_=maskw[:])
            nc.sync.dma_start(out=m2_t[:, FW:F2], in_=maskw[:])
            nc.scalar.dma_start(out=a_t[:], in_=a_c[:])
            nc.scalar.dma_start(out=id_t[:], in_=ident_c[:])

            # v = keep/drop code by mask: +code survives compaction.
            v_t = sp.tile([16, F2], mybir.dt.int16)
            nc.vector.tensor_copy(out=v_t[:], in_=b_t[:])
            nc.vector.copy_predicated(out=v_t[:], mask=m2_t[:], data=a_t[:])

            # Stable compaction: first L outputs = order[] encoded as code e+1.
            cidx_t = sp.tile([16, FW], mybir.dt.int16)
            nf_t = sp.tile([1, 1], mybir.dt.uint32)
            sg = nc.gpsimd.sparse_gather(cidx_t[:], v_t[:], num_found=nf_t[:])

            gcols = k128 // 16  # wrapped columns holding the first k128 slots

            # Codes to f32 for the PE transpose.
            cidxf_t = sp.tile([16, gcols], mybir.dt.float32)
            nc.vector.tensor_copy(out=cidxf_t[:], in_=cidx_t[:, 0:gcols])

            for g0 in range(0, gcols, 128):
                cols = min(128, gcols - g0)
                tok0 = g0 * 16
                # Natural order: psum[p, c] = code of output row tok0 + 16p + c.
                ps_t = pp.tile([128, 16], mybir.dt.float32)
                nc.tensor.transpose(
                    ps_t[:cols, :], cidxf_t[:, g0 : g0 + cols], id_t[:]
                )
                # Decode code w = pos+1 (boundary) or pos+4097 (tail).
                t1_t = sp.tile([128, 16], mybir.dt.float32)
                nc.vector.tensor_scalar(
                    t1_t[:cols], ps_t[:cols], 4097.0, -4096.0, Op.is_ge, Op.mult
                )
                idx_t = sp.tile([128, 16], mybir.dt.int32, tag=f"idx{g0}")
                nc.vector.scalar_tensor_tensor(
                    idx_t[:cols], t1_t[:cols], -1.0, ps_t[:cols], Op.add, Op.add
                )
                nm_t = sp.tile([128, 16], mybir.dt.uint8)
                nc.vector.tensor_scalar(
                    nm_t[:cols], ps_t[:cols], 4097.0, None, Op.is_lt
                )
                nc.sync.dma_start(
                    out=out_m[tok0 : tok0 + cols * 16].rearrange("(p c) -> p c", c=16),
                    in_=nm_t[:cols],
                )

                # Payload: one indirect DMA per token column (the DGE path
                # supports one dynamic offset per partition); store each
                # 4-column tile split across both HWDGE rings. Row 16p+c of
                # the output gets hidden[idx[p, c]], so each partition's two
                # store columns are 8KB contiguous in HBM.
                outg = out_h[tok0 : tok0 + cols * 16, :].rearrange(
                    "(p c) d -> p c d", c=16
                )
                for t in range(0, 16, 4):
                    pay_t = payp.tile([128, 4, D], mybir.dt.float32, tag="pay")
                    for cc in range(4):
                        nc.gpsimd.indirect_dma_start(
                            out=pay_t[:cols, cc, :],
                            out_offset=None,
                            in_=hidden[:],
                            in_offset=bass.IndirectOffsetOnAxis(
                                ap=idx_t[:cols, t + cc : t + cc + 1], axis=0
                            ),
                        )
                        seng = nc.sync if cc % 2 == 0 else nc.scalar
                        seng.dma_start(
                            out=outg[:, t + cc : t + cc + 1, :],
                            in_=pay_t[:cols, cc : cc + 1, :],
                        )

    nc.compile()
    return nc


@functools.lru_cache(maxsize=4)
def _built(k128: int):
    return _build(k128)


def _marshal_inputs(hidden_states, boundary_mask):
    """Per-core input dicts: row b of hidden + its wrapped mask."""
    in_maps = []
    for b in range(B):
        maskw = np.ascontiguousarray(
            boundary_mask[b].astype(np.uint8).reshape(FW, 16).T
        )
        in_maps.append(
            {
                "hidden": np.ascontiguousarray(hidden_states[b], dtype=np.float32),
                "maskw": maskw,
            }
        )
    return in_maps


def kernel(hidden_states, boundary_mask, mask, next_max_seqlen, _trace=False):
    from concourse.bass_utils import run_bass_kernel_spmd

    hidden_states = np.asarray(hidden_states)
    boundary_mask = np.asarray(boundary_mask)
    assert hidden_states.shape == (B, L, D), hidden_states.shape
    assert boundary_mask.shape == (B, L), boundary_mask.shape
    K = int(next_max_seqlen)
    assert 0 < K <= L
    k128 = ((K + 127) // 128) * 128

    nc = _built(k128)
    in_maps = _marshal_inputs(hidden_states, boundary_mask)
    res = run_bass_kernel_spmd(nc, in_maps, list(range(B)), trace=_trace)

    next_hidden = np.stack([res.results[b]["out_h"][:K] for b in range(B)])
    next_mask = np.stack([res.results[b]["out_m"][:K] for b in range(B)]).astype(bool)
    if _trace:
        return (next_hidden, next_mask), res
    return next_hidden, next_mask
